# revision 33
# baseline (speedup 1.0000x reference)
"""Longformer-style blocked local+global attention on 8 Trainium2 NeuronCores.

Problem (hardcoded): B=2, S=4096, D=1024, H=16, DH=64, W=256 (block size =
one-sided window radius), G=64 global prefix tokens.

Sharding: batch x head-group. Core c handles batch b = c//4 and heads
[(c%4)*4, (c%4)*4+4). Everything for one (b, head-group) is independent, so
there are no collectives; the only cross-core interaction is the final
output-projection partial sum, which the host performs (4 partials per
batch, written as bf16 and accumulated in f32 on host).

Per-core kernel layout strategy:
  - Host passes x[b] pre-transposed (xT = [D, S]) so all projection matmuls
    have their contraction dim (D) on SBUF partitions.
  - q, k are produced directly in transposed layout qk[hp] = [128, 2, S]
    (sub 0 = q, sub 1 = k for the head pair), one 8-matmul psum group +
    copy per (hp, q|k) "projection group".
  - v is produced in natural layout [S, DH] per head with a ones column at
    col 64 so PV accumulates the softmax denominator as psum row 64 for
    free; cols 65:128 are zero padding so the PV stationary is a full
    128-col weight load (65-col loads disable Fast Weight Load).
  - Scores are computed transposed (sT = [key_pos, query]).  The two heads
    of a pair occupy SBUF partitions 0:64 / 64:128, so their score matmuls
    auto-infer PE tiles (0,0)/(64,0) and run CONCURRENTLY in the two
    halves of the 128x128 array (~N cycles per pair, 2x the naive rate).
  - Global-prefix scores use a 128-key stationary (keys 64:128 are garbage
    rows) so they stay in the same 64x128 tile class and pair-overlap like
    the strip chunks; the garbage rows are cancelled in PV by vg, a copy
    of the global v padded with ZERO rows 64:128 (and zero ones-col rows),
    which also keeps the whole PV accumulation group in 128-row PE mode
    (64-row glb matmuls forced a PE tiling-mode switch mid-group and ran
    ~2x slow).
  - Softmax uses exp without max subtraction (scores are O(1) by
    construction); masked entries are zeroed after exp via precomputed 0/1
    triangle masks, duplicated per head-half so one strided DVE multiply
    covers both heads of a pair.
  - All matmul inputs are bf16; accumulation is fp32 in PSUM.  (fp8
    DoubleRow was evaluated and rejected: e4m3 quantization of any of
    x/W/et/v pushes rel err to 3-7e-2 against the 2e-2 gate.)

Scheduling (the PE HAM governor demotes the clock after idle and
duty-cycles ~3.4us of every ~31us under sustained load; the tensor stream
must never develop bubbles):
  - Startup: the first x k-tile (sync) and first two wq k-tiles (scalar)
    are tiny dedicated DMAs so the first projection matmul starts early;
    bulk weights stream on gpsimd and the mask tiles ride the otherwise
    idle sync queue behind chunk 0.
  - Blocks interleave with projection chunks: block n is emitted as soon
    as its q/k/v columns exist.  Per block: scores(hp0), scores(hp1),
    PV(hp0), PV(hp1).  emit_pv splits each head's accumulation into a
    head (c3,c2,glb,c1,c4 - needs only exps) and a tail (c0,c5 - also
    needs the ti2 combined mask), with covering PE work injected between
    hh0's head and tail: half of block n-1's Wo rides PV(hp0), the other
    half plus one qk projection group (even blocks) or one V-projection
    group (odd blocks) rides PV(hp1).  Two V groups also sit between the
    odd block's two score groups as an exp-drain window.
  - Score psum tiles go out c4/c3 first (c3 is the first chunk PV
    consumes); exps are per (tile, hh) so ACT drains each bank as soon as
    its concurrent matmul pair lands.
  - The denominator row bounces psum->SBUF on ACT (reciprocal needs exact
    f32 bits and DVE is the busier engine), reciprocal_approx_fast + the
    at_blk normalize multiplies run on DVE, the rec broadcast on Pool.
  - psum: score/projection/Wo tiles share one 6-bank rotation ("a" tag);
    the two PV accumulators double-buffer: 6 + 2 = all 8 banks.
"""

import numpy as np
import ml_dtypes

import concourse.bacc as bacc
import concourse.bass as bass
import concourse.mybir as mybir
import concourse.tile as tile
from concourse.bass_utils import run_bass_kernel_spmd

BF16 = mybir.dt.bfloat16
F32 = mybir.dt.float32
NPBF = ml_dtypes.bfloat16

B, S, D = 2, 4096, 1024
H, DH = 16, 64
W = 256          # block size == window radius
G = 64           # global prefix tokens
NB = S // W      # 16 blocks
SCALE = 1.0 / 8.0  # 1/sqrt(DH)

N_CORES = 8
HEADS_PER_CORE = 4
ECOLS = HEADS_PER_CORE * DH   # 256 embedding columns per core

# mask stack indices (each [128, 512] left-aligned, see build_masks)
M_L1, M_R0, M_EGEN, M_EN1, M_GC, M_R0R1, M_L0L1 = range(7)

# module-level caches
_BUILT = {}
LAST_RESULTS = None


def build_masks():
    """Returns (masks [7,128,512], cmasks [2,128,768]) bf16 0/1 tiles.

    Local-strip chunk c of query block n holds key rows kj of block n-1
    (c=0,1), n (c=2,3), n+1 (c=4,5). Triangle masks (r = row within chunk,
    q = query within block): c0: q<=r (only q<128 possible); c1: q<=128+r;
    c4: q>=r; c5: q>=128+r (only q>=128 possible).

    cmasks are the generic-block combined masks, one 256-col slab per score
    tile in emission order [t(c4|c3) | t(c1|c2) | t(c0|c5|glb)]:
    [R0 | L1 | L0h,R1h], with the n==1 variant applying the global cut to
    the c0 triangle.
    """
    r = np.arange(128)[:, None]
    q = np.arange(256)[None, :]
    L0 = (q <= r).astype(np.float32)          # use cols 0:128
    L1 = (q <= 128 + r).astype(np.float32)
    R0 = (q >= r).astype(np.float32)
    R1 = (q >= 128 + r).astype(np.float32)    # use cols 128:256
    L0g = L0 * (r >= G)                       # left-upper chunk w/ global cut
    Gc = np.broadcast_to((r >= G).astype(np.float32), (128, 256)).copy()

    L0h, L0gh, R1h = L0[:, 0:128], L0g[:, 0:128], R1[:, 128:256]

    def pad(*parts):
        m = np.concatenate(parts, axis=1)
        if m.shape[1] < 512:
            m = np.concatenate(
                [m, np.zeros((128, 512 - m.shape[1]), np.float32)], axis=1)
        return m

    m = np.stack([
        pad(L1),             # M_L1
        pad(R0),             # M_R0
        pad(L0h, R1h),       # M_EGEN  [c0|c5]
        pad(L0gh, R1h),      # M_EN1   [c0 w/ global cut|c5] (n==1)
        pad(Gc),             # M_GC    (n==0 c2)
        pad(R0, R1h),        # M_R0R1  (n==0 [c4|c5])
        pad(L0h, L1),        # M_L0L1  (n==15 [c0|c1])
    ]).astype(NPBF)

    egen = np.concatenate([L0h, R1h], axis=1)   # [128, 256]
    en1 = np.concatenate([L0gh, R1h], axis=1)
    cm = np.stack([
        np.stack([R0, L1, egen], axis=1),        # generic   [128, 3, 256]
        np.stack([R0, L1, en1], axis=1),         # n == 1
    ])                                           # [2, 128, 3, 256]
    cm2 = np.broadcast_to(cm[:, :, :, None, :],
                          (2, 128, 3, 2, 256)).astype(NPBF).copy()
    return m, cm2


def _tiles_for_block(n):
    """Per-head score-tile packing for query block n.

    Each head's active strip chunks (+ the global-prefix scores) are packed
    into three [128, 2, 512] double-bank PSUM tiles (bank = head-half hh).
    Returns (tiles, cmask_idx):
      parts: [(c, col0, width, q0)] -- c in 0..5 or 'glb'; the chunk's
             scores land at psum/et cols [col0, col0+width), covering query
             range [q0, q0+width)
      exps:  [(col0, col1)] -- merged exp ranges, applied across both hh
      mask:  (col0, col1, mask_idx) or None -- per-(hh) mask multiply (edge
             blocks only; generic blocks use the combined cmask instead)
    cmask_idx is None for edge blocks, else the cmasks row to apply over
    et[:, :, hh, 0:256].  Chunk c covers sequence tile s_tile = 2*(n-1)+c.
    Generic blocks emit the c4/c3 tile first (c3 is the first chunk PV
    consumes) with the masked chunk at cols 0:256 so the three masks line
    up for one strided multiply.
    """
    if n == 0:
        return [
            dict(parts=[(2, 0, 256, 0), (3, 256, 256, 0)],
                 exps=[(0, 512)], mask=(0, 256, M_GC)),
            dict(parts=[(4, 0, 256, 0), (5, 256, 128, 128)],
                 exps=[(0, 384)], mask=(0, 384, M_R0R1)),
            dict(parts=[("glb", 0, 256, 0)], exps=[(0, 256)], mask=None),
        ], None
    if n == NB - 1:
        return [
            dict(parts=[(0, 0, 128, 0), (1, 128, 256, 0)],
                 exps=[(0, 384)], mask=(0, 384, M_L0L1)),
            dict(parts=[(2, 0, 256, 0), (3, 256, 256, 0)],
                 exps=[(0, 512)], mask=None),
            dict(parts=[("glb", 0, 256, 0)], exps=[(0, 256)], mask=None),
        ], None
    return [
        dict(parts=[(4, 0, 256, 0), (3, 256, 256, 0)],
             exps=[(0, 512)], mask=None),
        dict(parts=[(1, 0, 256, 0), (2, 256, 256, 0)],
             exps=[(0, 512)], mask=None),
        dict(parts=[(0, 0, 128, 0), (5, 128, 128, 128), ("glb", 256, 256, 0)],
             exps=[(0, 512)], mask=None),
    ], (1 if n == 1 else 0)


def build():
    """Build the per-core Bass/Tile program (identical on all 8 cores)."""
    nc = bacc.Bacc("TRN2", target_bir_lowering=False, debug=False)

    xT = nc.dram_tensor("xT", [D, S], BF16, kind="ExternalInput")
    wq = nc.dram_tensor("wq", [D, ECOLS], BF16, kind="ExternalInput")
    wk = nc.dram_tensor("wk", [D, ECOLS], BF16, kind="ExternalInput")
    wv = nc.dram_tensor("wv", [D, ECOLS], BF16, kind="ExternalInput")
    wo = nc.dram_tensor("wo", [ECOLS, D], BF16, kind="ExternalInput")
    masks = nc.dram_tensor("masks", [7, 128, 512], BF16,
                           kind="ExternalInput")
    cmasks = nc.dram_tensor("cmasks", [2, 128, 3, 2, 256], BF16,
                            kind="ExternalInput")
    y = nc.dram_tensor("y", [S, D], BF16, kind="ExternalOutput")

    EXP = mybir.ActivationFunctionType.Exp
    COPY = mybir.ActivationFunctionType.Copy

    with tile.TileContext(nc) as tc:
        with (
            tc.tile_pool(name="const", bufs=1) as constp,
            tc.tile_pool(name="persist", bufs=1) as pers,
            tc.tile_pool(name="etp", bufs=4) as etp,
            tc.tile_pool(name="attnp", bufs=4) as atp,
            tc.tile_pool(name="smallp", bufs=8) as smp,
            tc.tile_pool(name="yp", bufs=2) as yp,
        ):
            # ---- constants ----
            wq_sb = constp.tile([128, 8, ECOLS], BF16, name="wq_sb")
            wk_sb = constp.tile([128, 8, ECOLS], BF16, name="wk_sb")
            wv_sb = constp.tile([128, 8, ECOLS], BF16, name="wv_sb")
            wo_sb = constp.tile([128, 2, D], BF16, name="wo_sb")
            mk_sb = constp.tile([128, 7, 512], BF16, name="mk_sb")
            cm_sb = constp.tile([128, 2, 3, 2, 256], BF16, name="cm_sb")

            def wq_part(k0, k1, eng):
                eng.dma_start(
                    out=wq_sb[:, k0:k1, :],
                    in_=wq.ap()[k0 * 128:k1 * 128, :]
                        .rearrange("(k p) e -> p k e", p=128))

            # tiny first piece on the scalar queue so the first projection
            # matmul starts as early as possible; bulk weights stream on
            # gpsimd; masks ride the sync queue (empty until y writes)
            wq_part(0, 2, nc.scalar)
            wq_part(2, 8, nc.gpsimd)
            nc.gpsimd.dma_start(
                out=wk_sb[:], in_=wk.ap().rearrange("(k p) e -> p k e", p=128))
            nc.gpsimd.dma_start(
                out=wv_sb[:], in_=wv.ap().rearrange("(k p) e -> p k e", p=128))
            nc.gpsimd.dma_start(
                out=wo_sb[:], in_=wo.ap().rearrange("(e p) d -> p e d", p=128))

            def load_masks():
                # on sync AFTER chunk 0's x pieces: the queue is otherwise
                # idle until the first y writes
                nc.sync.dma_start(
                    out=mk_sb[:], in_=masks.ap().rearrange("m p q -> p m q"))
                nc.sync.dma_start(
                    out=cm_sb[:],
                    in_=cmasks.ap().rearrange("g p t h q -> p g t h q"))

            # ---- persistent per-head tensors ----
            # qk[hp]: [128 dh-part, 2 (q|k), S]
            qk = [pers.tile([128, 2, S], BF16, name=f"qk{i}") for i in range(2)]
            # v natural layout: [128 seq-part, 32 seq-tiles, 4 heads, 65]
            # (col 64 = ones for the denominator row)
            # 65 live columns (64 v + ones) padded to 128 so the PV
            # stationary is a full 128-col weight load (enables FWL; a
            # 65-col load runs un-accelerated).  Pad columns are zeroed
            # once; psum rows 65:128 are written but never read.
            vv = pers.tile([128, S // 128, HEADS_PER_CORE, 128], BF16,
                           name="vv")
            for h in range(HEADS_PER_CORE):
                nc.vector.memset(vv[:, :, h, 64:65], 1.0)
                nc.vector.memset(vv[:, :, h, 65:128], 0.0)
            # vg: global-prefix v padded with ZERO rows 64:128 so the glb PV
            # matmul can run K=128 against exp'd-garbage et rows.
            vg = pers.tile([128, HEADS_PER_CORE, 128], BF16, name="vg")
            nc.vector.memset(vg[:], 0.0)
            nc.vector.memset(vg[0:64, :, 64:65], 1.0)

            # ---- interleaved projections + attention ----
            # Projection chunks and attention blocks share one PE stream so
            # the exp/mask/normalize engines (busy only during attention)
            # spread over the whole kernel instead of cramming into a
            # second phase: block n is emitted as soon as its q/k/v columns
            # (chunks <= (n+1)//2) exist.  All projection psum tiles join
            # the score/Wo rotation (tag "a"): 3 double-bank + 2 PV = 8.
            with (
                tc.tile_pool(name="xstream", bufs=3) as xp,
                tc.tile_pool(name="ps_s", bufs=6, space="PSUM") as ps_sp,
                tc.tile_pool(name="ps_o", bufs=2, space="PSUM") as ps_op,
            ):
                def start_chunk(c):
                    """Allocate + DMA the chunk's xT columns; return xt."""
                    xt = xp.tile([128, 8, 512], BF16, name="xt")
                    def xpart(k0, k1, eng):
                        eng.dma_start(
                            out=xt[:, k0:k1, :],
                            in_=xT.ap()[k0 * 128:k1 * 128,
                                        c * 512:(c + 1) * 512]
                                .rearrange("(k p) s -> p k s", p=128))
                    if c == 0:
                        # tiny first piece so the first matmul starts early
                        xpart(0, 1, nc.sync)
                        xpart(1, 4, nc.sync)
                        xpart(4, 6, nc.scalar)
                        xpart(6, 8, nc.scalar)
                    else:
                        xpart(0, 4, nc.sync)
                        xpart(4, 8, nc.scalar)
                    return xt

                def qk_group(c, xt, g):
                    """One (hp, q|k) projection group: 8 matmuls + copy."""
                    hp, j = g // 2, g % 2
                    wsb = (wq_sb, wk_sb)[j]
                    ps = ps_sp.tile([128, 512], F32, name="ps_qk", tag="a")
                    for k in range(8):
                        nc.tensor.matmul(
                            ps[:],
                            wsb[:, k, hp * 128:(hp + 1) * 128],
                            xt[:, k, :],
                            start=(k == 0), stop=(k == 7))
                    if j == 1:
                        # split the k evacuation: cols 0:128 are the first
                        # keys the next block's score LDWEIGHTS waits on
                        nc.vector.tensor_copy(
                            qk[hp][:, j, c * 512:c * 512 + 128],
                            ps[:, 0:128])
                        nc.vector.tensor_copy(
                            qk[hp][:, j, c * 512 + 128:(c + 1) * 512],
                            ps[:, 128:512])
                    else:
                        # split at the query-block boundary: block 2c's
                        # scores stream cols 0:256 first
                        nc.vector.tensor_copy(
                            qk[hp][:, j, c * 512:c * 512 + 256],
                            ps[:, 0:256])
                        nc.vector.tensor_copy(
                            qk[hp][:, j, c * 512 + 256:(c + 1) * 512],
                            ps[:, 256:512])

                def v_group(c, xt, ss):
                    """One 128-row seq subtile of the V projection."""
                    ps = ps_sp.tile([128, 512], F32, name="ps_v", tag="a")
                    for k in range(8):
                        nc.tensor.matmul(
                            ps[:, 0:ECOLS],
                            xt[:, k, ss * 128:(ss + 1) * 128],
                            wv_sb[:, k, :],
                            start=(k == 0), stop=(k == 7))
                    nc.vector.tensor_copy(
                        vv[:, c * 4 + ss, :, 0:64],
                        ps[:, 0:ECOLS].rearrange("p (h e) -> p h e", h=4))
                    if c == 0 and ss == 0:
                        # global v rows (seq 0:64) into zero-padded vg
                        nc.vector.tensor_copy(
                            vg[0:64, :, 0:64],
                            ps[0:64, 0:ECOLS]
                            .rearrange("p (h e) -> p h e", h=4))

                def emit_scores(n, hp, tiles, cidx, et, loc):
                    qp = qk[hp]
                    for ti, sp in enumerate(tiles):
                        sts = []
                        for hh in range(2):
                            sts.append(ps_sp.tile([128, 512], F32, name="st",
                                                  tag="a"))
                        for c, col0, width, q0 in sp["parts"]:
                            for hh in range(2):
                                hr = hh * 64
                                if c == "glb":
                                    # padded to 128 keys: rows 64:128 are
                                    # garbage, cancelled by vg's zero rows
                                    lhs = qp[hr:hr + 64, 1, 0:128]
                                else:
                                    s0 = (2 * (n - 1) + c) * 128
                                    lhs = qp[hr:hr + 64, 1, s0:s0 + 128]
                                nc.tensor.matmul(
                                    sts[hh][:, col0:col0 + width],
                                    lhs,
                                    qp[hr:hr + 64, 0,
                                       n * 256 + q0:n * 256 + q0 + width],
                                    start=True, stop=True)
                            loc[c] = (ti, col0, width, q0)
                        for hh in range(2):
                            for c0e, c1e in sp["exps"]:
                                nc.scalar.activation(
                                    et[:, ti, hh, c0e:c1e],
                                    sts[hh][:, c0e:c1e], EXP, scale=SCALE)
                        if sp["mask"] is not None:  # edge blocks
                            m0, m1, mi = sp["mask"]
                            for hh in range(2):
                                nc.vector.tensor_mul(
                                    et[:, ti, hh, m0:m1],
                                    et[:, ti, hh, m0:m1],
                                    mk_sb[:, mi, 0:m1 - m0])
                    if cidx is not None:
                        # strided multiplies cover the tiles' triangle
                        # masks (both head-halves per op); ti0/ti1 split
                        # from ti2 so PV's c1/c4 chunks aren't gated on
                        # the last exp
                        nc.vector.tensor_mul(
                            et[:, 0:2, :, 0:256],
                            et[:, 0:2, :, 0:256],
                            cm_sb[:, cidx, 0:2, :, :])
                        nc.vector.tensor_mul(
                            et[:, 2, :, 0:256],
                            et[:, 2, :, 0:256],
                            cm_sb[:, cidx, 2, :, :])

                def emit_pv(n, hp, et, loc, at_blk, late=None):
                    # PV + normalize; the two heads share one psum bank and
                    # their accumulation groups must stay sequential (a
                    # group start clears has_written for the whole bank).
                    # Chunk order: early chunks need only the exps (glb
                    # included -- it dodges the combined mask), the (c0,c5)
                    # tail also needs the ti2 combined mask, so late() (Wo
                    # or projection groups) is inserted before hh0's tail
                    # to cover the exp+mask chain latency.
                    pv_head = [c for c in (3, 2, "glb", 1, 4) if c in loc]
                    pv_tail = [c for c in (0, 5) if c in loc]
                    ot = ps_op.tile([128, 512], F32, name="ot", tag="ot")

                    def chunk_mm(hh, c, start, stop):
                        h = hp * 2 + hh
                        ob = hh * 256
                        ti, col0, width, q0 = loc[c]
                        if c == "glb":
                            # K=128 against vg (zero rows 64:128) keeps the
                            # PE in 128-row mode for the whole group
                            stat = vg[:, h, :]
                        else:
                            stat = vv[:, 2 * (n - 1) + c, h, :]
                        nc.tensor.matmul(
                            ot[:, ob + q0:ob + q0 + width],
                            stat,
                            et[:, ti, hh, col0:col0 + width],
                            start=start, stop=stop)

                    for i, c in enumerate(pv_head):
                        chunk_mm(0, c, i == 0, False)
                    if late is not None:
                        late()
                    for i, c in enumerate(pv_tail):
                        chunk_mm(0, c, False, i == len(pv_tail) - 1)
                    for i, c in enumerate(pv_head):
                        chunk_mm(1, c, i == 0, False)
                    for i, c in enumerate(pv_tail):
                        chunk_mm(1, c, False, i == len(pv_tail) - 1)

                    # normalize: reciprocal_approx_fast needs exact fp32
                    # bits; its PSUM read path perturbs them (HW-measured
                    # ~5% error), so bounce the den row through SBUF -- on
                    # ACT, keeping DVE free for the masks and copies.
                    den = smp.tile([1, 512], F32, name="den")
                    nc.scalar.activation(den[:], ot[64:65, 0:512], COPY)
                    rec = smp.tile([1, 512], F32, name="rec")
                    nc.vector.reciprocal_approx_fast(rec[:], den[:])
                    recb = smp.tile([64, 512], F32, name="recb")
                    nc.gpsimd.partition_broadcast(recb[:], rec[:])
                    for hh in range(2):
                        ob = hh * 256
                        nc.vector.tensor_mul(
                            at_blk[hh * 64:(hh + 1) * 64, hp, :],
                            ot[0:64, ob:ob + 256], recb[:, ob:ob + 256])

                def emit_wo(n, at_blk, only_ss=None):
                    """Output projection for block n's 256 rows."""
                    for ss in ((0, 1) if only_ss is None else (only_ss,)):
                        ysb = yp.tile([128, D], BF16, name="ysb")
                        for dk in range(2):
                            py_ = ps_sp.tile([128, 512], F32, name="py",
                                             tag="a")
                            for e in range(2):
                                nc.tensor.matmul(
                                    py_[:],
                                    at_blk[:, e, ss * 128:(ss + 1) * 128],
                                    wo_sb[:, e, dk * 512:(dk + 1) * 512],
                                    start=(e == 0), stop=(e == 1))
                            # partials are bf16 (host accumulates in f32;
                            # costs ~2e-3 rel and halves the tail DMA).
                            # The last blocks have no projection interleave
                            # and DVE backlogs there, so their copies go to
                            # ACT instead.
                            if n >= 13:
                                nc.scalar.activation(
                                    ysb[:, dk * 512:(dk + 1) * 512],
                                    py_[:], COPY)
                            else:
                                nc.vector.tensor_copy(
                                    ysb[:, dk * 512:(dk + 1) * 512], py_[:])
                        r0 = n * 256 + ss * 128
                        nc.sync.dma_start(out=y.ap()[r0:r0 + 128, :],
                                          in_=ysb[:])

                # Wo for block n runs between block n+1's two PVs so the
                # in-order PE never stalls on the normalize chain and the
                # second PV's exps/masks get extra cover.
                state = {"pending": None}

                def emit_block(n, mid=None, late=None):
                    at_blk = atp.tile([128, 2, 256], BF16, name="at_blk")
                    state["cur_at"] = at_blk
                    tiles, cidx = _tiles_for_block(n)
                    loc = {}
                    ets = []
                    for hp in range(2):
                        et = etp.tile([128, 3, 2, 512], BF16, name="et")
                        ets.append(et)
                        emit_scores(n, hp, tiles, cidx, et, loc)
                        if hp == 0 and mid is not None:
                            mid()  # chunk V matmuls drain hp0's exps
                    wo0 = wo1 = None
                    if state["pending"] is not None:
                        pend = state["pending"]
                        wo0 = lambda: emit_wo(*pend, only_ss=0)
                        wo1 = lambda: emit_wo(*pend, only_ss=1)
                    def late1():
                        if wo1 is not None:
                            wo1()
                        if late is not None:
                            late()
                    emit_pv(n, 0, ets[0], loc, at_blk, late=wo0)
                    emit_pv(n, 1, ets[1], loc, at_blk, late=late1)
                    state["pending"] = (n, at_blk)

                # block n's inputs exist once chunk (n+1)//2 is in.
                # Projection groups are spread through the blocks: one qk
                # group lands inside the even block's PV-hp1 (tail cover),
                # two v groups sit between the odd block's score groups
                # (exp-drain window) and one inside its PV-hp1.
                xt0 = start_chunk(0)
                for g in range(4):
                    qk_group(0, xt0, g)
                for ss in range(4):
                    v_group(0, xt0, ss)
                xt1 = start_chunk(1)
                load_masks()  # behind chunk 1's x so xt1 isn't delayed
                emit_block(0, late=lambda: qk_group(1, xt1, 0))
                for g in range(1, 4):
                    qk_group(1, xt1, g)
                emit_block(1,
                           mid=lambda: [v_group(1, xt1, 0),
                                        v_group(1, xt1, 1)],
                           late=lambda: v_group(1, xt1, 2))
                v_group(1, xt1, 3)
                for c in range(2, 8):
                    xt = start_chunk(c)
                    # qk groups ride the even block (two between the score
                    # groups, one inside PV-hp1) so the chunk's k/q copies
                    # land well before block 2c-1's scores consume them
                    emit_block(2 * c - 2,
                               late=lambda xt=xt, c=c: qk_group(c, xt, 0))
                    for g in range(1, 4):
                        qk_group(c, xt, g)
                    emit_block(2 * c - 1,
                               mid=lambda xt=xt, c=c: [v_group(c, xt, 0),
                                                       v_group(c, xt, 1)],
                               late=lambda xt=xt, c=c: v_group(c, xt, 2))
                    v_group(c, xt, 3)
                emit_block(14)
                # final block: Wo's e0 half rides inside PV(15,hp1) so the
                # last norm chain overlaps PE work; e1 + evacuation follow
                fin = {}

                def wo15_e0():
                    at15 = state["cur_at"]
                    for ss in range(2):
                        for dk in range(2):
                            py_ = ps_sp.tile([128, 512], F32, name="py",
                                             tag="a")
                            nc.tensor.matmul(
                                py_[:],
                                at15[:, 0, ss * 128:(ss + 1) * 128],
                                wo_sb[:, 0, dk * 512:(dk + 1) * 512],
                                start=True, stop=False)
                            fin[(ss, dk)] = py_

                emit_block(15, late=wo15_e0)
                n15, at15 = state["pending"]
                for ss in range(2):
                    ysb = yp.tile([128, D], BF16, name="ysb")
                    for dk in range(2):
                        py_ = fin[(ss, dk)]
                        nc.tensor.matmul(
                            py_[:],
                            at15[:, 1, ss * 128:(ss + 1) * 128],
                            wo_sb[:, 1, dk * 512:(dk + 1) * 512],
                            start=False, stop=True)
                        # ACT: the end-phase DVE queue is the tail's
                        # critical path (masks/norms for blocks 14-15)
                        nc.scalar.activation(
                            ysb[:, dk * 512:(dk + 1) * 512], py_[:], COPY)
                    r0 = n15 * 256 + ss * 128
                    nc.sync.dma_start(out=y.ap()[r0:r0 + 128, :],
                                      in_=ysb[:])

    nc.compile()
    return nc


def _get_nc():
    if "nc" not in _BUILT:
        _BUILT["nc"] = build()
    return _BUILT["nc"]


def make_in_maps(x, Wq, Wk, Wv, Wo):
    masks_np, cmasks_np = build_masks()
    xT = [np.ascontiguousarray(x[b].T).astype(NPBF) for b in range(B)]
    wq16, wk16, wv16 = (w.astype(NPBF) for w in (Wq, Wk, Wv))
    wo16 = Wo.astype(NPBF)
    in_maps = []
    for core in range(N_CORES):
        b, hg = core // 4, core % 4
        cols = slice(hg * ECOLS, (hg + 1) * ECOLS)
        in_maps.append({
            "xT": xT[b],
            "wq": np.ascontiguousarray(wq16[:, cols]),
            "wk": np.ascontiguousarray(wk16[:, cols]),
            "wv": np.ascontiguousarray(wv16[:, cols]),
            "wo": np.ascontiguousarray(wo16[cols, :]),
            "masks": masks_np,
            "cmasks": cmasks_np,
        })
    return in_maps


def kernel(x, Wq, Wk, Wv, Wo):
    global LAST_RESULTS
    nc = _get_nc()
    in_maps = make_in_maps(x, Wq, Wk, Wv, Wo)
    res = run_bass_kernel_spmd(nc, in_maps, core_ids=list(range(N_CORES)))
    LAST_RESULTS = res
    out = np.zeros((B, S, D), np.float32)
    for core in range(N_CORES):
        out[core // 4] += res.results[core]["y"].astype(np.float32)
    return out
